# revision 30
# baseline (speedup 1.0000x reference)
"""Cross-attention (GQA + RoPE) Trainium2 Bass kernel — v7.

Sharding: 8 cores = 4 batches x 2 head-groups (column-parallel QKV,
row-parallel w_out; host sums the two partials per batch).

Optimizations over the original baseline:
  * Host-side kv compaction: the reference gives masked kv positions
    EXACTLY zero probability (exp underflow), so only the unmasked kv
    columns (padded to 128) are shipped/computed: 16 -> 9 chunks.
  * bf16 operands (fp32 PSUM accumulation), rel-err ~6e-3 << 2e-2.
  * Host tensors pre-arranged partition-major per tile so every DMA is
    128 large contiguous descriptors; issue spread over 4 engine rings
    (~100 GB/s per ring, serial within a ring).
  * ALL PSUM tiles are single-bank (512 wide). exp instruction overhead
    is ~18ns so 512-wide exp is free, and the 4-deep transient PSUM
    ring lets Q-projection (next block) and out-projection (previous
    block) matmuls interleave INSIDE the attention chunk loops. A
    dense PE stream keeps the activity monitor hot (the PE downclocks
    ~2x when the queue has embedded waits - that, not peak rate, was
    the previous bottleneck).
  * reciprocal_approx_fast + bf16 ones-broadcast matmul for softmax
    normalization; norm flushes ride inside the NEXT pair's chunks.

Per-core resident layout (feature/head_dim on partitions):
  q_blk[b,h] [128, 8, 512]  query^T tile, partition-major
  kv_blk[kt] [128, 8, KW]   compacted key_value^T tile
  wq [128,8,512] wk/wv [128,8,128] wout [128,4,1024]  (head-PERMUTED:
    pair tile j = local heads (j, j+4))
  cosQ/sinQ [128, TQ], cosK/sinK [128, TKVC] rope tables (rows
    [c;c;c;c] / [-s;s;-s;s]; K tables gathered at kept positions)
  maskb [128, NCH]  additive bias per chunk: 0 real / -30000 pad
  Kt [128, TKVC] bf16 rope'd K^T; Vt[2] [128, 65*NCH] V + ones-col

Per (block, pair, chunk, half):
  scores^T [128kv, 512] = Kt_c.T @ Qt_half   (1-bank PSUM)
  e = exp(0.125*s^T + bias)                  (ACT, bf16 out)
  acc[head][half] [65, 512] += Vt_c.T @ e    (row 64 = denominator)
norm (per half): U=copy(acc), inv=rcp_fast(den), ps_b=ones^T@inv_bf,
  attnT = U * ps_b  (flushed during the next pair's chunks)
out: out[128 rows, 512] partial = attnT.T @ wout -> SBUF -> DMA (fp32)
"""

import os
from collections import deque
from contextlib import ExitStack

import numpy as np
import ml_dtypes

import concourse.bass as bass
import concourse.bacc as bacc
import concourse.mybir as mybir
import concourse.tile as tile
from concourse.bass_utils import run_bass_kernel_spmd

F32 = mybir.dt.float32
BF16 = mybir.dt.bfloat16

D_MODEL = 1024
N_HEADS = 16
NUM_KV_HEADS = 4
D_K = 64
ROPE_BASE = 10000.0
TQ = 2048
NEG_BIAS = -30000.0


def _kv_tile_width(tkv_c):
    """Largest multiple-of-128 divisor of tkv_c that is <= 512."""
    nch = tkv_c // 128
    for d in (4, 3, 2, 1):
        if nch % d == 0:
            return d * 128
    return 128


def build_bass(tq=TQ, tkv_c=1152, t2=1024):
    """Single-core SPMD program; tkv_c = compacted kv length (mult of 128)."""
    nc = bacc.Bacc("TRN2", target_bir_lowering=False, debug=False)
    P = 128
    NCH = tkv_c // 128
    NT2 = tq // t2
    NPAIR = 4
    KW = _kv_tile_width(tkv_c)
    NKT = tkv_c // KW
    NQB = tq // 512

    qT = nc.dram_tensor("qT", [P, NQB * 8 * 512], BF16, kind="ExternalInput").ap()
    kvT = nc.dram_tensor("kvT", [P, NKT * 8 * KW], BF16, kind="ExternalInput").ap()
    wq = nc.dram_tensor("wq", [P, 8 * 512], BF16, kind="ExternalInput").ap()
    wk = nc.dram_tensor("wk", [P, 8 * 128], BF16, kind="ExternalInput").ap()
    wv = nc.dram_tensor("wv", [P, 8 * 128], BF16, kind="ExternalInput").ap()
    wout = nc.dram_tensor("wout", [P, 4 * D_MODEL], BF16, kind="ExternalInput").ap()
    cosQ = nc.dram_tensor("cosQ", [P, tq], F32, kind="ExternalInput").ap()
    sinQ = nc.dram_tensor("sinQ", [P, tq], F32, kind="ExternalInput").ap()
    cosK = nc.dram_tensor("cosK", [P, tkv_c], F32, kind="ExternalInput").ap()
    sinK = nc.dram_tensor("sinK", [P, tkv_c], F32, kind="ExternalInput").ap()
    maskb = nc.dram_tensor("maskb", [P, NCH], F32, kind="ExternalInput").ap()
    onesb = nc.dram_tensor("onesb", [P, 64], BF16, kind="ExternalInput").ap()
    out = nc.dram_tensor("out", [tq, D_MODEL], F32, kind="ExternalOutput").ap()

    with tile.TileContext(nc) as tc, ExitStack() as ctx:
        const = ctx.enter_context(tc.tile_pool(name="const", bufs=1))
        kvp = ctx.enter_context(tc.tile_pool(name="kvp", bufs=2))
        qbp = ctx.enter_context(tc.tile_pool(name="qbp", bufs=4))
        qpool = ctx.enter_context(tc.tile_pool(name="qpool", bufs=1))
        apool = ctx.enter_context(tc.tile_pool(name="apool", bufs=1))
        workp = ctx.enter_context(tc.tile_pool(name="workp", bufs=4))
        ropep = ctx.enter_context(tc.tile_pool(name="ropep", bufs=2))
        outp = ctx.enter_context(tc.tile_pool(name="outp", bufs=3))
        psp = ctx.enter_context(tc.tile_pool(name="psp", bufs=4, space="PSUM"))

        def PS(name):
            return psp.tile([P, 512], F32, tag="ps", name=name)

        def MM(out_ap, lhsT, rhs, start, stop, chain=None):
            inst = nc.tensor.matmul(out_ap, lhsT, rhs, start=start, stop=stop)
            if chain is not None:
                tc.chain_iter_dep(chain, inst.ins)
            return inst

        def chain_dve(inst):
            tc.chain_iter_dep("dve_norm", inst.ins)
            return inst

        # ---- constant loads, spread over 4 issue rings ---------------------------
        # gpsimd: K-projection path (wk + kv tiles + wv, in the kv loop below)
        wk_sb = const.tile([P, 8, 128], BF16)
        nc.gpsimd.dma_start(out=wk_sb, in_=wk.rearrange("p (c f) -> p c f", c=8))
        # scalar: small early tensors (K rope tables, mask, ones columns);
        # the scalar engine is idle until the first exp (~30us in)
        cosK_sb = const.tile([P, tkv_c], F32)
        nc.scalar.dma_start(out=cosK_sb, in_=cosK)
        sinK_sb = const.tile([P, tkv_c], F32)
        nc.scalar.dma_start(out=sinK_sb, in_=sinK)
        mask_sb = const.tile([P, NCH], F32)
        nc.scalar.dma_start(out=mask_sb, in_=maskb)
        ones_bf = const.tile([1, 64], BF16)
        nc.scalar.dma_start(out=ones_bf, in_=onesb[0:1, :])
        Kt = const.tile([P, tkv_c], BF16)
        Vt = [const.tile([P, NCH * 65], BF16, name=f"Vt{i}") for i in range(2)]
        for i in range(2):
            nc.scalar.dma_start(
                out=Vt[i].rearrange("p (c k) -> p c k", k=65)[:, :, 64],
                in_=onesb[:, :NCH],
            )
        # sync: Q-projection path
        wq_sb = const.tile([P, 8, 512], BF16)
        nc.sync.dma_start(out=wq_sb, in_=wq.rearrange("p (c f) -> p c f", c=8))
        qT4 = qT.rearrange("p (b c t) -> p b c t", b=NQB, c=8)
        q_blks = {}
        for qb_i in range(NQB):
            qb = qbp.tile([P, 8, 512], BF16, tag="qb", name="q_blk")
            nc.sync.dma_start(out=qb, in_=qT4[:, qb_i])
            q_blks[qb_i] = qb
        # scalar: Q rope tables + wout (first needed ~25us in)
        cosQ_sb = const.tile([P, tq], F32)
        nc.scalar.dma_start(out=cosQ_sb, in_=cosQ)
        sinQ_sb = const.tile([P, tq], F32)
        nc.scalar.dma_start(out=sinQ_sb, in_=sinQ)
        wout_sb = const.tile([P, 4, D_MODEL], BF16)
        nc.scalar.dma_start(out=wout_sb, in_=wout.rearrange("p (c f) -> p c f", c=4))

        def rope_apply(dest, ps, cos_sb, sin_sb, col0, width):
            """dest[128, width] = rope(ps[128, width] PSUM) for positions
            col0..col0+width. Rows: two stacked heads, each [x1(32); x2(32)]."""
            cs = cos_sb[:, col0 : col0 + width]
            t_cos = ropep.tile([P, 512], F32, tag="rope", name="t_cos")
            t_u = ropep.tile([P, 512], F32, tag="rope", name="t_u")
            tc_ = t_cos[:, :width]
            tu_ = t_u[:, :width]
            nc.vector.tensor_mul(tc_, ps, cs)
            for b0 in (0, 64):
                # sin rows [b0:b0+32] = -sin, [b0+32:b0+64] = +sin
                nc.vector.tensor_mul(
                    tu_[b0 : b0 + 32, :],
                    ps[b0 + 32 : b0 + 64, :],
                    sin_sb[b0 : b0 + 32, col0 : col0 + width],
                )
                nc.vector.tensor_mul(
                    tu_[b0 + 32 : b0 + 64, :],
                    ps[b0 : b0 + 32, :],
                    sin_sb[b0 + 32 : b0 + 64, col0 : col0 + width],
                )
            nc.vector.tensor_add(dest, tc_, tu_)

        # ---- phase KV: K/V projections -------------------------------------------
        kvT4 = kvT.rearrange("p (kt c t) -> p kt c t", kt=NKT, c=8)
        wv_loaded = False
        for kt in range(NKT):
            off = kt * KW
            kv_blk = kvp.tile([P, 8, KW], BF16, tag="kv", name="kv_blk")
            nc.gpsimd.dma_start(out=kv_blk, in_=kvT4[:, kt])
            if not wv_loaded:
                wv_sb = const.tile([P, 8, 128], BF16)
                nc.gpsimd.dma_start(
                    out=wv_sb, in_=wv.rearrange("p (c f) -> p c f", c=8)
                )
                wv_loaded = True
            ps_k = PS("ps_k")
            for d in range(8):
                MM(ps_k[:, :KW], wk_sb[:, d, :], kv_blk[:, d, :], d == 0, d == 7)
            rope_apply(Kt[:, off : off + KW], ps_k[:, :KW], cosK_sb, sinK_sb, off, KW)
            for s in range(KW // 128):
                ps_v = PS("ps_v")
                pv = ps_v[:, 0:128]
                c = off // 128 + s
                for d in range(8):
                    MM(
                        pv,
                        kv_blk[:, d, s * 128 : (s + 1) * 128],
                        wv_sb[:, d, :],
                        d == 0,
                        d == 7,
                    )
                nc.vector.tensor_copy(
                    out=Vt[0][:, c * 65 : c * 65 + 64], in_=pv[:, 0:64]
                )
                nc.vector.tensor_copy(
                    out=Vt[1][:, c * 65 : c * 65 + 64], in_=pv[:, 64:128]
                )

        # ---- Q projection (one (pair, half) group at a time) ---------------------
        Qt = {
            (it2, j): qpool.tile(
                [P, t2], BF16, tag=f"Q{it2}{j}", name=f"Qt{it2}{j}"
            )
            for it2 in range(NT2)
            for j in range(NPAIR)
        }
        attnT = [
            [
                apool.tile([P, t2], BF16, tag=f"A{it2}{j}", name=f"attnT{it2}{j}")
                for j in range(NPAIR)
            ]
            for it2 in range(NT2)
        ]

        def emit_qproj(it2, j, half):
            hs = slice(half * 512, (half + 1) * 512)
            ps_q = PS("ps_q")
            for d in range(8):
                MM(
                    ps_q,
                    wq_sb[:, d, j * 128 : (j + 1) * 128],
                    q_blks[it2 * (t2 // 512) + half][:, d, :],
                    d == 0,
                    d == 7,
                    chain="pe_attn",
                )
            rope_apply(
                Qt[(it2, j)][:, hs], ps_q, cosQ_sb, sinQ_sb, it2 * t2 + half * 512, 512
            )

        ob_tiles = {}

        def emit_outproj(it2, s, n):
            """Half of one 128-row slice of block it2's output projection."""
            if n == 0:
                ob_tiles[(it2, s)] = outp.tile([P, D_MODEL], F32, tag="ob", name="ob")
            ps_f = PS("ps_f")
            for p_ in range(NPAIR):
                MM(
                    ps_f,
                    attnT[it2][p_][:, s * 128 : (s + 1) * 128],
                    wout_sb[:, p_, n * 512 : (n + 1) * 512],
                    p_ == 0,
                    p_ == NPAIR - 1,
                    chain="pe_attn",
                )
            ob = ob_tiles[(it2, s)]
            nc.vector.tensor_copy(out=ob[:, n * 512 : (n + 1) * 512], in_=ps_f)
            if n == 1:
                r0 = it2 * t2 + s * 128
                nc.sync.dma_start(out=out[r0 : r0 + 128, :], in_=ob)

        pending = []  # (it2, j, base, half, U, inv_bf) normalizations

        def flush_norm():
            if not pending:
                return
            it2_, j_, base_, half_, U_, invbf_ = pending.pop(0)
            hs = slice(half_ * 512, (half_ + 1) * 512)
            ps_b = PS("ps_b")
            MM(ps_b[0:64, :], ones_bf, invbf_, True, True, chain="pe_attn")
            chain_dve(
                nc.vector.tensor_mul(
                    attnT[it2_][j_][base_ : base_ + 64, hs],
                    U_,
                    ps_b[0:64, :],
                )
            )

        # fillers: PE work groups that ride inside the attention chunk loops
        fillers = deque()
        for j in range(NPAIR):
            for half in range(2):
                fillers.append((lambda j=j, h=half: emit_qproj(1, j, h)))
        # (outproj fillers for block 0 are appended once block 0 finishes)

        # Q projection for block 0 runs up front
        for j in range(NPAIR):
            for half in range(2):
                emit_qproj(0, j, half)

        # ---- attention -----------------------------------------------------------
        for it2 in range(NT2):
            for j in range(NPAIR):
                heads = [(j, 0, 0), (j + 4, 1, 64)]  # (head, kvh, base)
                # per (head, half) accumulators, single-bank each
                ps_os = {
                    (ab, half): psp.tile(
                        [65, 512], F32, tag="acc", name=f"ps_o{ab}{half}"
                    )
                    for ab in range(2)
                    for half in range(2)
                }

                def emit_pv(c_, exs_):
                    for ab in range(2):
                        kvh = heads[ab][1]
                        for half in range(2):
                            MM(
                                ps_os[(ab, half)],
                                Vt[kvh][:, c_ * 65 : c_ * 65 + 65],
                                exs_[(ab, half)],
                                c_ == 0,
                                c_ == NCH - 1,
                                chain="pe_attn",
                            )

                prev = None
                for c in range(NCH):
                    exs = {}
                    for ab in range(2):
                        base = heads[ab][2]
                        for half in range(2):
                            ps_s = PS("ps_s")
                            MM(
                                ps_s,
                                Kt[base : base + 64, c * 128 : (c + 1) * 128],
                                Qt[(it2, j)][
                                    base : base + 64, half * 512 : (half + 1) * 512
                                ],
                                True,
                                True,
                                chain="pe_attn",
                            )
                            ex = workp.tile(
                                [P, 512], BF16, tag="expT", name="ex", bufs=8
                            )
                            nc.scalar.activation(
                                out=ex,
                                in_=ps_s,
                                func=mybir.ActivationFunctionType.Exp,
                                bias=mask_sb[:, c : c + 1],
                                scale=0.125,
                            )
                            exs[(ab, half)] = ex
                    if prev is not None:
                        emit_pv(c - 1, prev)
                    prev = exs
                    if c % 2 == 1:
                        flush_norm()  # previous pair's norms (4 per pair)
                        # fillers ride along (not in the very first pair:
                        # block-1 q tiles are still in flight on DMA then)
                        if (it2, j) != (0, 0):
                            if fillers and c % 4 == 3:
                                fillers.popleft()()
                            elif len(fillers) > 4 and c % 4 == 1:
                                fillers.popleft()()
                emit_pv(NCH - 1, prev)

                # Accumulator copies FIRST (release PSUM promptly on the
                # in-order DVE), then the reciprocal chain per (head, half).
                Us = {}
                for ab in range(2):
                    for half in range(2):
                        U = workp.tile([64, 512], F32, tag="unorm", name="U", bufs=8)
                        chain_dve(
                            nc.vector.tensor_copy(
                                out=U, in_=ps_os[(ab, half)][0:64, :]
                            )
                        )
                        Us[(ab, half)] = U
                for ab in range(2):
                    base = heads[ab][2]
                    for half in range(2):
                        den = workp.tile([1, 512], F32, tag="den", name="den", bufs=2)
                        chain_dve(
                            nc.vector.tensor_copy(
                                out=den, in_=ps_os[(ab, half)][64:65, :]
                            )
                        )
                        inv = workp.tile([1, 512], F32, tag="inv", name="inv", bufs=2)
                        chain_dve(nc.vector.reciprocal_approx_fast(out=inv, in_=den))
                        inv_bf = workp.tile(
                            [1, 512], BF16, tag="invbf", name="inv_bf", bufs=8
                        )
                        chain_dve(nc.vector.tensor_copy(out=inv_bf, in_=inv))
                        pending.append((it2, j, base, half, Us[(ab, half)], inv_bf))

                # After pair 0 of block b, every norm of block b-1 has been
                # flushed -> its out-projection may now ride as fillers.
                if j == 0 and it2 > 0:
                    for s in range(t2 // 128):
                        for n in range(2):
                            fillers.append(
                                (lambda i=it2 - 1, s=s, n=n: emit_outproj(i, s, n))
                            )

        # tail: flush remaining norms, then the last block's out-projection
        while pending:
            flush_norm()
        while fillers:
            fillers.popleft()()
        for s in range(t2 // 128):
            for n in range(2):
                emit_outproj(NT2 - 1, s, n)

    nc.compile()
    return nc


# ---------------------------------------------------------------------------
# host-side sharding / prep
# ---------------------------------------------------------------------------

_HEAD_PERM = [0, 4, 1, 5, 2, 6, 3, 7]  # local head order inside pair tiles


def _rope_tables(n):
    theta = ROPE_BASE ** (-np.arange(0, D_K, 2, dtype=np.float32) / D_K)  # [32]
    pos = np.arange(n, dtype=np.float32)[:, None]
    ang = pos * theta[None, :]  # [n,32]
    c = np.cos(ang).T.astype(np.float32)  # [32, n]
    s = np.sin(ang).T.astype(np.float32)
    cosF = np.concatenate([c, c, c, c], axis=0)
    sinF = np.concatenate([-s, s, -s, s], axis=0)
    return np.ascontiguousarray(cosF), np.ascontiguousarray(sinF)


def _bf16(x):
    return np.ascontiguousarray(x.astype(ml_dtypes.bfloat16))


def _pmajor(a, c):
    """[c*128, f] -> partition-major [128, c*f] (row r=c_i*128+p -> [p, c_i, :])."""
    f = a.shape[1]
    return a.reshape(c, 128, f).transpose(1, 0, 2).reshape(128, c * f)


def make_in_maps(query, key_value, kv_mask, w_q, w_k, w_v, w_out, tq, tkv_c):
    nb = query.shape[0]
    tkv = key_value.shape[1]
    cosF, sinF = _rope_tables(max(tq, tkv))
    NCH = tkv_c // 128
    col_perm = np.concatenate(
        [np.arange(h * D_K, (h + 1) * D_K) for h in _HEAD_PERM]
    )
    onesb = np.ones((128, 64), np.float32)
    in_maps = []
    for core in range(2 * nb):
        b = core // 2
        g = core % 2
        idx = np.flatnonzero(kv_mask[b])
        n_b = len(idx)
        kv_c = np.zeros((tkv_c, D_MODEL), np.float32)
        kv_c[:n_b] = key_value[b][idx]
        cosK = np.zeros((128, tkv_c), np.float32)
        sinK = np.zeros((128, tkv_c), np.float32)
        cosK[:, :n_b] = cosF[:, idx]
        sinK[:, :n_b] = sinF[:, idx]
        mb = np.full(tkv_c, NEG_BIAS, np.float32)
        mb[:n_b] = 0.0
        maskb = np.ascontiguousarray(mb.reshape(NCH, 128).T)
        wq_g = w_q[:, g * 512 : (g + 1) * 512][:, col_perm]
        # per-tile layouts: kvT[p, kt, c, t'] = kv_c[kt*KW+t', c*128+p];
        # qT[p, blk, c, t'] = query[b][blk*512+t', c*128+p]
        KW = _kv_tile_width(tkv_c)
        kv_t = kv_c.reshape(tkv_c // KW, KW, 8, 128).transpose(3, 0, 2, 1)
        q_t = query[b].reshape(tq // 512, 512, 8, 128).transpose(3, 0, 2, 1)
        in_maps.append(
            {
                "qT": _bf16(q_t.reshape(128, -1)),
                "kvT": _bf16(kv_t.reshape(128, -1)),
                "wq": _bf16(_pmajor(wq_g, 8)),
                "wk": _bf16(_pmajor(w_k[:, g * 128 : (g + 1) * 128], 8)),
                "wv": _bf16(_pmajor(w_v[:, g * 128 : (g + 1) * 128], 8)),
                "wout": _bf16(
                    _pmajor(w_out[g * 512 : (g + 1) * 512, :][col_perm, :], 4)
                ),
                "cosQ": np.ascontiguousarray(cosF[:, :tq]),
                "sinQ": np.ascontiguousarray(sinF[:, :tq]),
                "cosK": cosK,
                "sinK": sinK,
                "maskb": maskb,
                "onesb": _bf16(onesb),
            }
        )
    return in_maps


_NC_CACHE = {}


def _get_nc(tq, tkv_c):
    key = (tq, tkv_c)
    if key not in _NC_CACHE:
        _NC_CACHE[key] = build_bass(tq, tkv_c)
    return _NC_CACHE[key]


def _run(inputs, trace=False):
    query = np.asarray(inputs["query"], dtype=np.float32)
    key_value = np.asarray(inputs["key_value"], dtype=np.float32)
    kv_mask = np.asarray(inputs["kv_mask"])
    w_q = np.asarray(inputs["w_q"], dtype=np.float32)
    w_k = np.asarray(inputs["w_k"], dtype=np.float32)
    w_v = np.asarray(inputs["w_v"], dtype=np.float32)
    w_out = np.asarray(inputs["w_out"], dtype=np.float32)
    nb, tq, _ = query.shape

    tkv_c = max(256, int(-(-int(kv_mask.sum(axis=1).max()) // 128)) * 128)
    nc = _get_nc(tq, tkv_c)
    in_maps = make_in_maps(query, key_value, kv_mask, w_q, w_k, w_v, w_out, tq, tkv_c)
    res = run_bass_kernel_spmd(
        nc, in_maps, list(range(2 * nb)), trace=trace, trace_cores=[0]
    )
    outs = [np.asarray(r["out"]) for r in res.results]
    full = np.stack([outs[2 * b] + outs[2 * b + 1] for b in range(nb)])

    query_mask = np.asarray(inputs["query_mask"])
    if not query_mask.all():
        # masked query rows: reference yields uniform attention over all kv
        for b in range(nb):
            rows = ~query_mask[b]
            if rows.any():
                V = key_value[b] @ w_v  # [tkv, 256]
                meanV = V.mean(axis=0)  # [256]
                group = N_HEADS // NUM_KV_HEADS
                feat = np.concatenate([meanV.reshape(NUM_KV_HEADS, D_K)[h // group]
                                       for h in range(N_HEADS)])
                full[b, rows, :] = feat @ w_out
    return full.astype(np.float32), res


def kernel(**inputs):
    out, _ = _run(inputs, trace=False)
    return out


def kernel_traced(**inputs):
    out, res = _run(inputs, trace=True)
    return out, res


if __name__ == "__main__":
    print("kernel.py is a library; use test.py")


# revision 34
# speedup vs baseline: 1.0040x; 1.0040x over previous
"""Cross-attention (GQA + RoPE) Trainium2 Bass kernel — v7.

Sharding: 8 cores = 4 batches x 2 head-groups (column-parallel QKV,
row-parallel w_out; host sums the two partials per batch).

Optimizations over the original baseline:
  * Host-side kv compaction: the reference gives masked kv positions
    EXACTLY zero probability (exp underflow), so only the unmasked kv
    columns (padded to 128) are shipped/computed: 16 -> 9 chunks.
  * bf16 operands (fp32 PSUM accumulation), rel-err ~6e-3 << 2e-2.
  * Host tensors pre-arranged partition-major per tile so every DMA is
    128 large contiguous descriptors; issue spread over 4 engine rings
    (~100 GB/s per ring, serial within a ring).
  * ALL PSUM tiles are single-bank (512 wide). exp instruction overhead
    is ~18ns so 512-wide exp is free, and the 4-deep transient PSUM
    ring lets Q-projection (next block) and out-projection (previous
    block) matmuls interleave INSIDE the attention chunk loops. A
    dense PE stream keeps the activity monitor hot (the PE downclocks
    ~2x when the queue has embedded waits - that, not peak rate, was
    the previous bottleneck).
  * reciprocal_approx_fast + bf16 ones-broadcast matmul for softmax
    normalization; norm flushes ride inside the NEXT pair's chunks.

Per-core resident layout (feature/head_dim on partitions):
  q_blk[b,h] [128, 8, 512]  query^T tile, partition-major
  kv_blk[kt] [128, 8, KW]   compacted key_value^T tile
  wq [128,8,512] wk/wv [128,8,128] wout [128,4,1024]  (head-PERMUTED:
    pair tile j = local heads (j, j+4))
  cosQ/sinQ [128, TQ], cosK/sinK [128, TKVC] rope tables (rows
    [c;c;c;c] / [-s;s;-s;s]; K tables gathered at kept positions)
  maskb [128, NCH]  additive bias per chunk: 0 real / -30000 pad
  Kt [128, TKVC] bf16 rope'd K^T; Vt[2] [128, 65*NCH] V + ones-col

Per (block, pair, chunk, half):
  scores^T [128kv, 512] = Kt_c.T @ Qt_half   (1-bank PSUM)
  e = exp(0.125*s^T + bias)                  (ACT, bf16 out)
  acc[head][half] [65, 512] += Vt_c.T @ e    (row 64 = denominator)
norm (per half): U=copy(acc), inv=rcp_fast(den), ps_b=ones^T@inv_bf,
  attnT = U * ps_b  (flushed during the next pair's chunks)
out: out[128 rows, 512] partial = attnT.T @ wout -> SBUF -> DMA (fp32)
"""

import os
from contextlib import ExitStack

import numpy as np
import ml_dtypes

import concourse.bass as bass
import concourse.bacc as bacc
import concourse.mybir as mybir
import concourse.tile as tile
from concourse.bass_utils import run_bass_kernel_spmd

F32 = mybir.dt.float32
BF16 = mybir.dt.bfloat16

D_MODEL = 1024
N_HEADS = 16
NUM_KV_HEADS = 4
D_K = 64
ROPE_BASE = 10000.0
TQ = 2048
NEG_BIAS = -30000.0


def _kv_tile_width(tkv_c):
    """Largest multiple-of-128 divisor of tkv_c that is <= 512."""
    nch = tkv_c // 128
    for d in (4, 3, 2, 1):
        if nch % d == 0:
            return d * 128
    return 128


def build_bass(tq=TQ, tkv_c=1152, t2=1024):
    """Single-core SPMD program; tkv_c = compacted kv length (mult of 128)."""
    nc = bacc.Bacc("TRN2", target_bir_lowering=False, debug=False)
    P = 128
    NCH = tkv_c // 128
    NT2 = tq // t2
    NPAIR = 4
    KW = _kv_tile_width(tkv_c)
    NKT = tkv_c // KW
    NQB = tq // 512

    qT = nc.dram_tensor("qT", [P, NQB * 8 * 512], BF16, kind="ExternalInput").ap()
    kvT = nc.dram_tensor("kvT", [P, NKT * 8 * KW], BF16, kind="ExternalInput").ap()
    wq = nc.dram_tensor("wq", [P, 8 * 512], BF16, kind="ExternalInput").ap()
    wk = nc.dram_tensor("wk", [P, 8 * 128], BF16, kind="ExternalInput").ap()
    wv = nc.dram_tensor("wv", [P, 8 * 128], BF16, kind="ExternalInput").ap()
    wout = nc.dram_tensor("wout", [P, 4 * D_MODEL], BF16, kind="ExternalInput").ap()
    cosQ = nc.dram_tensor("cosQ", [P, tq], F32, kind="ExternalInput").ap()
    sinQ = nc.dram_tensor("sinQ", [P, tq], F32, kind="ExternalInput").ap()
    cosK = nc.dram_tensor("cosK", [P, tkv_c], F32, kind="ExternalInput").ap()
    sinK = nc.dram_tensor("sinK", [P, tkv_c], F32, kind="ExternalInput").ap()
    maskb = nc.dram_tensor("maskb", [P, NCH], F32, kind="ExternalInput").ap()
    onesb = nc.dram_tensor("onesb", [P, 64], BF16, kind="ExternalInput").ap()
    out = nc.dram_tensor("out", [tq, D_MODEL], F32, kind="ExternalOutput").ap()

    with tile.TileContext(nc) as tc, ExitStack() as ctx:
        const = ctx.enter_context(tc.tile_pool(name="const", bufs=1))
        kvp = ctx.enter_context(tc.tile_pool(name="kvp", bufs=2))
        qbp = ctx.enter_context(tc.tile_pool(name="qbp", bufs=4))
        qpool = ctx.enter_context(tc.tile_pool(name="qpool", bufs=1))
        apool = ctx.enter_context(tc.tile_pool(name="apool", bufs=1))
        workp = ctx.enter_context(tc.tile_pool(name="workp", bufs=4))
        ropep = ctx.enter_context(tc.tile_pool(name="ropep", bufs=2))
        outp = ctx.enter_context(tc.tile_pool(name="outp", bufs=3))
        psp = ctx.enter_context(tc.tile_pool(name="psp", bufs=2, space="PSUM"))

        def PS(name):
            return psp.tile([P, t2], F32, tag="sps", name=name)

        def MM(out_ap, lhsT, rhs, start, stop, chain=None):
            inst = nc.tensor.matmul(out_ap, lhsT, rhs, start=start, stop=stop)
            if chain is not None:
                tc.chain_iter_dep(chain, inst.ins)
            return inst

        def chain_dve(inst):
            tc.chain_iter_dep("dve_norm", inst.ins)
            return inst

        # ---- constant loads, spread over 4 issue rings ---------------------------
        # gpsimd: K-projection path (wk + kv tiles + wv, in the kv loop below)
        wk_sb = const.tile([P, 8, 128], BF16)
        nc.gpsimd.dma_start(out=wk_sb, in_=wk.rearrange("p (c f) -> p c f", c=8))
        # scalar: small early tensors (K rope tables, mask, ones columns);
        # the scalar engine is idle until the first exp (~30us in)
        cosK_sb = const.tile([P, tkv_c], F32)
        nc.scalar.dma_start(out=cosK_sb, in_=cosK)
        sinK_sb = const.tile([P, tkv_c], F32)
        nc.scalar.dma_start(out=sinK_sb, in_=sinK)
        mask_sb = const.tile([P, NCH], F32)
        nc.scalar.dma_start(out=mask_sb, in_=maskb)
        ones_bf = const.tile([1, 64], BF16)
        nc.scalar.dma_start(out=ones_bf, in_=onesb[0:1, :])
        Kt = const.tile([P, tkv_c], BF16)
        Vt = [const.tile([P, NCH * 65], BF16, name=f"Vt{i}") for i in range(2)]
        for i in range(2):
            nc.scalar.dma_start(
                out=Vt[i].rearrange("p (c k) -> p c k", k=65)[:, :, 64],
                in_=onesb[:, :NCH],
            )
        # sync: Q-projection path
        wq_sb = const.tile([P, 8, 512], BF16)
        nc.sync.dma_start(out=wq_sb, in_=wq.rearrange("p (c f) -> p c f", c=8))
        qT4 = qT.rearrange("p (b c t) -> p b c t", b=NQB, c=8)
        q_blks = {}
        for qb_i in range(NQB):
            qb = qbp.tile([P, 8, 512], BF16, tag="qb", name="q_blk")
            nc.sync.dma_start(out=qb, in_=qT4[:, qb_i])
            q_blks[qb_i] = qb
        # scalar: Q rope tables + wout (first needed ~25us in)
        cosQ_sb = const.tile([P, tq], F32)
        nc.scalar.dma_start(out=cosQ_sb, in_=cosQ)
        sinQ_sb = const.tile([P, tq], F32)
        nc.scalar.dma_start(out=sinQ_sb, in_=sinQ)
        wout_sb = const.tile([P, 4, D_MODEL], BF16)
        nc.scalar.dma_start(out=wout_sb, in_=wout.rearrange("p (c f) -> p c f", c=4))

        def rope_apply(dest, ps, cos_sb, sin_sb, col0, width):
            """dest[128, width] = rope(ps[128, width] PSUM) for positions
            col0..col0+width. Rows: two stacked heads, each [x1(32); x2(32)]."""
            cs = cos_sb[:, col0 : col0 + width]
            t_cos = ropep.tile([P, t2], F32, tag="rope", name="t_cos")
            t_u = ropep.tile([P, t2], F32, tag="rope", name="t_u")
            tc_ = t_cos[:, :width]
            tu_ = t_u[:, :width]
            nc.vector.tensor_mul(tc_, ps, cs)
            for b0 in (0, 64):
                # sin rows [b0:b0+32] = -sin, [b0+32:b0+64] = +sin
                nc.vector.tensor_mul(
                    tu_[b0 : b0 + 32, :],
                    ps[b0 + 32 : b0 + 64, :],
                    sin_sb[b0 : b0 + 32, col0 : col0 + width],
                )
                nc.vector.tensor_mul(
                    tu_[b0 + 32 : b0 + 64, :],
                    ps[b0 : b0 + 32, :],
                    sin_sb[b0 + 32 : b0 + 64, col0 : col0 + width],
                )
            nc.vector.tensor_add(dest, tc_, tu_)

        # ---- phase KV: K/V projections -------------------------------------------
        kvT4 = kvT.rearrange("p (kt c t) -> p kt c t", kt=NKT, c=8)
        wv_loaded = False
        for kt in range(NKT):
            off = kt * KW
            kv_blk = kvp.tile([P, 8, KW], BF16, tag="kv", name="kv_blk")
            nc.gpsimd.dma_start(out=kv_blk, in_=kvT4[:, kt])
            if not wv_loaded:
                wv_sb = const.tile([P, 8, 128], BF16)
                nc.gpsimd.dma_start(
                    out=wv_sb, in_=wv.rearrange("p (c f) -> p c f", c=8)
                )
                wv_loaded = True
            ps_k = PS("ps_k")
            for d in range(8):
                MM(ps_k[:, :KW], wk_sb[:, d, :], kv_blk[:, d, :], d == 0, d == 7)
            rope_apply(Kt[:, off : off + KW], ps_k[:, :KW], cosK_sb, sinK_sb, off, KW)
            for s in range(KW // 128):
                ps_v = PS("ps_v")
                pv = ps_v[:, 0:128]
                c = off // 128 + s
                for d in range(8):
                    MM(
                        pv,
                        kv_blk[:, d, s * 128 : (s + 1) * 128],
                        wv_sb[:, d, :],
                        d == 0,
                        d == 7,
                    )
                nc.vector.tensor_copy(
                    out=Vt[0][:, c * 65 : c * 65 + 64], in_=pv[:, 0:64]
                )
                nc.vector.tensor_copy(
                    out=Vt[1][:, c * 65 : c * 65 + 64], in_=pv[:, 64:128]
                )

        # ---- phase Q: all (block, pair) projections up front ---------------------
        Qt = {
            (it2, j): qpool.tile(
                [P, t2], BF16, tag=f"Q{it2}{j}", name=f"Qt{it2}{j}"
            )
            for it2 in range(NT2)
            for j in range(NPAIR)
        }
        attnT = [
            [
                apool.tile([P, t2], BF16, tag=f"A{it2}{j}", name=f"attnT{it2}{j}")
                for j in range(NPAIR)
            ]
            for it2 in range(NT2)
        ]

        for it2 in range(NT2):
            for j in range(NPAIR):
                ps_q = PS("ps_q")
                for half in range(2):
                    for d in range(8):
                        MM(
                            ps_q[:, half * 512 : (half + 1) * 512],
                            wq_sb[:, d, j * 128 : (j + 1) * 128],
                            q_blks[it2 * (t2 // 512) + half][:, d, :],
                            d == 0,
                            d == 7,
                        )
                rope_apply(Qt[(it2, j)], ps_q, cosQ_sb, sinQ_sb, it2 * t2, t2)

        # ---- attention + output projection ---------------------------------------
        ob_tiles = {}

        def emit_outproj(it2, s, n):
            """Half of one 128-row slice of block it2's output projection."""
            if n == 0:
                ob_tiles[(it2, s)] = outp.tile([P, D_MODEL], F32, tag="ob", name="ob")
            ps_f = PS("ps_f")
            pf = ps_f[:, 0:512]
            for p_ in range(NPAIR):
                MM(
                    pf,
                    attnT[it2][p_][:, s * 128 : (s + 1) * 128],
                    wout_sb[:, p_, n * 512 : (n + 1) * 512],
                    p_ == 0,
                    p_ == NPAIR - 1,
                    chain="pe_attn",
                )
            ob = ob_tiles[(it2, s)]
            nc.vector.tensor_copy(out=ob[:, n * 512 : (n + 1) * 512], in_=pf)
            if n == 1:
                r0 = it2 * t2 + s * 128
                nc.sync.dma_start(out=out[r0 : r0 + 128, :], in_=ob)

        pending = []  # (it2, j, base, U, inv_bf) normalizations to flush

        def flush_norm():
            """Emit one pending head's broadcast matmul + normalize mul."""
            if not pending:
                return
            it2_, j_, base_, U_, invbf_ = pending.pop(0)
            ps_b = PS("ps_b")
            for half in range(2):
                hs = slice(half * 512, (half + 1) * 512)
                MM(ps_b[0:64, hs], ones_bf, invbf_[:, hs], True, True,
                   chain="pe_attn")
            chain_dve(
                nc.vector.tensor_mul(
                    attnT[it2_][j_][base_ : base_ + 64, :],
                    U_,
                    ps_b[0:64, :],
                )
            )

        for it2 in range(NT2):
            for j in range(NPAIR):
                heads = [(j, 0, 0), (j + 4, 1, 64)]  # (head, kvh, base)
                ps_os = [
                    psp.tile([65, t2], F32, tag="acc", name=f"ps_o{ab}")
                    for ab in range(2)
                ]

                def emit_pv(c_, exs_):
                    for ab in range(2):
                        kvh = heads[ab][1]
                        for half in range(2):
                            MM(
                                ps_os[ab][:, half * 512 : (half + 1) * 512],
                                Vt[kvh][:, c_ * 65 : c_ * 65 + 65],
                                exs_[ab][:, half * 512 : (half + 1) * 512],
                                c_ == 0,
                                c_ == NCH - 1,
                                chain="pe_attn",
                            )

                # PV lags scores by one chunk; the previous pair's norms
                # flush at chunks 2 and 5.
                prev = None
                for c in range(NCH):
                    exs = []
                    for ab in range(2):
                        base = heads[ab][2]
                        ps_s = PS("ps_s")
                        for half in range(2):
                            MM(
                                ps_s[:, half * 512 : (half + 1) * 512],
                                Kt[base : base + 64, c * 128 : (c + 1) * 128],
                                Qt[(it2, j)][
                                    base : base + 64, half * 512 : (half + 1) * 512
                                ],
                                True,
                                True,
                                chain="pe_attn",
                            )
                        ex = workp.tile([P, t2], BF16, tag="expT", name="ex", bufs=4)
                        nc.scalar.activation(
                            out=ex,
                            in_=ps_s,
                            func=mybir.ActivationFunctionType.Exp,
                            bias=mask_sb[:, c : c + 1],
                            scale=0.125,
                        )
                        exs.append(ex)
                    if prev is not None:
                        emit_pv(c - 1, prev)
                    prev = exs
                    if c in (2, 5):
                        flush_norm()
                emit_pv(NCH - 1, prev)

                # out-proj of the previous block rides along at pair end.
                if it2 > 0:
                    for n in range(2):
                        emit_outproj(it2 - 1, 2 * j, n)
                        emit_outproj(it2 - 1, 2 * j + 1, n)

                # Accumulator copies on the (otherwise idle) gpsimd engine:
                # releases both PSUM slots without clogging the in-order DVE
                # at the pair boundary. Only the approx reciprocal runs on
                # the DVE.
                Us = []
                for ab in range(2):
                    U = workp.tile([64, t2], F32, tag="unorm", name="U", bufs=4)
                    chain_dve(nc.vector.tensor_copy(out=U, in_=ps_os[ab][0:64, :]))
                    Us.append(U)
                for ab in range(2):
                    base = heads[ab][2]
                    den = workp.tile([1, t2], F32, tag="den", name="den", bufs=2)
                    chain_dve(nc.vector.tensor_copy(out=den, in_=ps_os[ab][64:65, :]))
                    inv = workp.tile([1, t2], F32, tag="inv", name="inv", bufs=2)
                    chain_dve(nc.vector.reciprocal_approx_fast(out=inv, in_=den))
                    inv_bf = workp.tile([1, t2], BF16, tag="invbf", name="inv_bf", bufs=4)
                    nc.gpsimd.tensor_copy(out=inv_bf, in_=inv)
                    pending.append((it2, j, base, Us[ab], inv_bf))

        # tail: flush remaining norms, then the last block's out-projection
        while pending:
            flush_norm()
        for s in range(t2 // 128):
            for n in range(2):
                emit_outproj(NT2 - 1, s, n)

    nc.compile()
    return nc


# ---------------------------------------------------------------------------
# host-side sharding / prep
# ---------------------------------------------------------------------------

_HEAD_PERM = [0, 4, 1, 5, 2, 6, 3, 7]  # local head order inside pair tiles


def _rope_tables(n):
    theta = ROPE_BASE ** (-np.arange(0, D_K, 2, dtype=np.float32) / D_K)  # [32]
    pos = np.arange(n, dtype=np.float32)[:, None]
    ang = pos * theta[None, :]  # [n,32]
    c = np.cos(ang).T.astype(np.float32)  # [32, n]
    s = np.sin(ang).T.astype(np.float32)
    cosF = np.concatenate([c, c, c, c], axis=0)
    sinF = np.concatenate([-s, s, -s, s], axis=0)
    return np.ascontiguousarray(cosF), np.ascontiguousarray(sinF)


def _bf16(x):
    return np.ascontiguousarray(x.astype(ml_dtypes.bfloat16))


def _pmajor(a, c):
    """[c*128, f] -> partition-major [128, c*f] (row r=c_i*128+p -> [p, c_i, :])."""
    f = a.shape[1]
    return a.reshape(c, 128, f).transpose(1, 0, 2).reshape(128, c * f)


def make_in_maps(query, key_value, kv_mask, w_q, w_k, w_v, w_out, tq, tkv_c):
    nb = query.shape[0]
    tkv = key_value.shape[1]
    cosF, sinF = _rope_tables(max(tq, tkv))
    NCH = tkv_c // 128
    col_perm = np.concatenate(
        [np.arange(h * D_K, (h + 1) * D_K) for h in _HEAD_PERM]
    )
    onesb = np.ones((128, 64), np.float32)
    in_maps = []
    for core in range(2 * nb):
        b = core // 2
        g = core % 2
        idx = np.flatnonzero(kv_mask[b])
        n_b = len(idx)
        kv_c = np.zeros((tkv_c, D_MODEL), np.float32)
        kv_c[:n_b] = key_value[b][idx]
        cosK = np.zeros((128, tkv_c), np.float32)
        sinK = np.zeros((128, tkv_c), np.float32)
        cosK[:, :n_b] = cosF[:, idx]
        sinK[:, :n_b] = sinF[:, idx]
        mb = np.full(tkv_c, NEG_BIAS, np.float32)
        mb[:n_b] = 0.0
        maskb = np.ascontiguousarray(mb.reshape(NCH, 128).T)
        wq_g = w_q[:, g * 512 : (g + 1) * 512][:, col_perm]
        # per-tile layouts: kvT[p, kt, c, t'] = kv_c[kt*KW+t', c*128+p];
        # qT[p, blk, c, t'] = query[b][blk*512+t', c*128+p]
        KW = _kv_tile_width(tkv_c)
        kv_t = kv_c.reshape(tkv_c // KW, KW, 8, 128).transpose(3, 0, 2, 1)
        q_t = query[b].reshape(tq // 512, 512, 8, 128).transpose(3, 0, 2, 1)
        in_maps.append(
            {
                "qT": _bf16(q_t.reshape(128, -1)),
                "kvT": _bf16(kv_t.reshape(128, -1)),
                "wq": _bf16(_pmajor(wq_g, 8)),
                "wk": _bf16(_pmajor(w_k[:, g * 128 : (g + 1) * 128], 8)),
                "wv": _bf16(_pmajor(w_v[:, g * 128 : (g + 1) * 128], 8)),
                "wout": _bf16(
                    _pmajor(w_out[g * 512 : (g + 1) * 512, :][col_perm, :], 4)
                ),
                "cosQ": np.ascontiguousarray(cosF[:, :tq]),
                "sinQ": np.ascontiguousarray(sinF[:, :tq]),
                "cosK": cosK,
                "sinK": sinK,
                "maskb": maskb,
                "onesb": _bf16(onesb),
            }
        )
    return in_maps


_NC_CACHE = {}


def _get_nc(tq, tkv_c):
    key = (tq, tkv_c)
    if key not in _NC_CACHE:
        _NC_CACHE[key] = build_bass(tq, tkv_c)
    return _NC_CACHE[key]


def _run(inputs, trace=False):
    query = np.asarray(inputs["query"], dtype=np.float32)
    key_value = np.asarray(inputs["key_value"], dtype=np.float32)
    kv_mask = np.asarray(inputs["kv_mask"])
    w_q = np.asarray(inputs["w_q"], dtype=np.float32)
    w_k = np.asarray(inputs["w_k"], dtype=np.float32)
    w_v = np.asarray(inputs["w_v"], dtype=np.float32)
    w_out = np.asarray(inputs["w_out"], dtype=np.float32)
    nb, tq, _ = query.shape

    tkv_c = max(256, int(-(-int(kv_mask.sum(axis=1).max()) // 128)) * 128)
    nc = _get_nc(tq, tkv_c)
    in_maps = make_in_maps(query, key_value, kv_mask, w_q, w_k, w_v, w_out, tq, tkv_c)
    res = run_bass_kernel_spmd(
        nc, in_maps, list(range(2 * nb)), trace=trace, trace_cores=[0]
    )
    outs = [np.asarray(r["out"]) for r in res.results]
    full = np.stack([outs[2 * b] + outs[2 * b + 1] for b in range(nb)])

    query_mask = np.asarray(inputs["query_mask"])
    if not query_mask.all():
        # masked query rows: reference yields uniform attention over all kv
        for b in range(nb):
            rows = ~query_mask[b]
            if rows.any():
                V = key_value[b] @ w_v  # [tkv, 256]
                meanV = V.mean(axis=0)  # [256]
                group = N_HEADS // NUM_KV_HEADS
                feat = np.concatenate([meanV.reshape(NUM_KV_HEADS, D_K)[h // group]
                                       for h in range(N_HEADS)])
                full[b, rows, :] = feat @ w_out
    return full.astype(np.float32), res


def kernel(**inputs):
    out, _ = _run(inputs, trace=False)
    return out


def kernel_traced(**inputs):
    out, res = _run(inputs, trace=True)
    return out, res


if __name__ == "__main__":
    print("kernel.py is a library; use test.py")
